# revision 45
# baseline (speedup 1.0000x reference)
"""Trainium2 Bass kernel for nn_MultiHeadedAttention (B=2, H=16, S=2048, d=64).

Sharding: data-parallel over batch x tensor-parallel over heads.
8 cores = 2 batch groups x 4 head-groups (4 heads each).

Per core (batch b, 4 heads as 2 head-pairs hp):
  - bf16 projections: qT/kT = W^T @ x^T in [head-dim, seq] layout, v in
    [seq, head-dim] layout with a ones column for the softmax denominator.
  - chunk-major block-causal attention: per 512-wide q-chunk, all k-strips
    run before the next chunk (so a phase can start on one projection tile
    and eager normalizations fire early).  Both heads' score matmuls use
    disjoint PE row groups (rows 0-63 / 64-127) and run concurrently; one
    ScalarE exp op covers both (scale=1/8, no max subtraction: max causal
    score ~7.4 so exp is safe; masked entries exactly 0 like the f32
    reference where exp(-10000-max) underflows).  PV matmuls accumulate
    hs_nat[q, 65] in PSUM (ones column -> denominator), then reciprocal +
    broadcast-multiply normalization.  No zero-prefill: the first PV write
    per bank per phase uses start=True (bank-wide has_written clear), so
    all other slots' first writes overwrite-on-clear.
  - PE-transpose hs -> hsT, out_partial = hsT^T @ Wo_rows; O-proj and the
    hp1 transposes interleave into the attention phases as exp-wait filler.
Ramp/clock-gate management (the HAM throttles the PE to 1.2GHz without
~3.4us of continuous activity, and re-throttles on idle windows):
  - 32 dep-free full-K warm-up matmuls run first (K=1 warm-ups don't
    register in the activity monitor),
  - the first q/k projections run double-buffered in the boot PSUM pool so
    the 8-MM accumulation groups don't bubble on PSUM-evacuation reads,
  - input DMAs are priority-ordered in waves across both HWDGE rings
    (sync + scalar), weights host-preshuffled to partition-contiguous
    layouts for large descriptors; wave 1 carries exactly what the q/k
    projections (and hence the critical ScalarE exp chain) need.
Output is written bf16 (halves the 8MB/core drain; host sums partials in
f64), the last tiles' DMA pieces are split across both rings post-exp.
No GpSimd use at all (avoids the Q7 library-load stall).
Host: shards/transposes inputs, sums the 4 partial outputs per batch, adds
the (b_V @ W_O + b_O) row (exact because softmax rows sum to 1).
"""

import math
from contextlib import ExitStack

import numpy as np
import ml_dtypes

import concourse.bass as bass
import concourse.mybir as mybir
import concourse.tile as tile
from concourse import bacc, bass_utils

F32 = mybir.dt.float32
F32R = mybir.dt.float32r
BF16 = mybir.dt.bfloat16
EXP = mybir.ActivationFunctionType.Exp

B, S, D = 2, 2048, 1024
NH, HD = 16, 64
NCORES = 8
GROUPS = NCORES // B          # 4 head-groups per batch
HPC = NH // GROUPS            # 4 heads per core
M = HPC * HD                  # 256 local head-dims per core
P = 128
KC = D // P                   # 8 contraction chunks
NT = S // P                   # 16 q/s tiles
SCALE = 1.0 / math.sqrt(HD)   # 0.125


def build_kernel():
    nc = bacc.Bacc("TRN2", target_bir_lowering=False)

    # wq/wk/wv/wo are host-preshuffled to [128, 2048] with each partition's
    # data contiguous in DRAM (4KB rows) so DMA descriptors are large; free
    # dim is kc-major (matches the [P, KC, M] SBUF layout directly).
    xT_d = nc.dram_tensor("xT", [D, S], BF16, kind="ExternalInput")
    wq_d = nc.dram_tensor("wq", [P, KC * M], BF16, kind="ExternalInput")
    wk_d = nc.dram_tensor("wk", [P, KC * M], BF16, kind="ExternalInput")
    wv_d = nc.dram_tensor("wv", [P, KC * M], BF16, kind="ExternalInput")
    wo_d = nc.dram_tensor("wo", [P, 2 * D], BF16, kind="ExternalInput")
    bq_d = nc.dram_tensor("bq", [M], F32, kind="ExternalInput")
    bk_d = nc.dram_tensor("bk", [M], F32, kind="ExternalInput")
    tri_d = nc.dram_tensor("tri", [P, P], BF16, kind="ExternalInput")
    ident_d = nc.dram_tensor("ident", [P, P], BF16, kind="ExternalInput")
    out_d = nc.dram_tensor("out", [S, D], BF16, kind="ExternalOutput")

    with tile.TileContext(nc) as tc, ExitStack() as ctx:
        big = ctx.enter_context(tc.tile_pool(name="big", bufs=1))
        exp_pool = ctx.enter_context(tc.tile_pool(name="expp", bufs=12))
        outcp = ctx.enter_context(tc.tile_pool(name="outcp", bufs=4))
        recip_pool = ctx.enter_context(tc.tile_pool(name="recipp", bufs=2))

        # ---- persistent SBUF tiles ----
        xT_sb = big.tile([P, KC, S], BF16)
        wq_sb = big.tile([P, KC, M], BF16)
        wk_sb = big.tile([P, KC, M], BF16)
        wv_sb = big.tile([P, KC, M], BF16)
        wo_sb = big.tile([P, 2, D], BF16)
        bq_sb = big.tile([P, 2], F32)
        bk_sb = big.tile([P, 2], F32)
        qT_sb = big.tile([P, 2, S], BF16)
        kT_sb = big.tile([P, 2, S], BF16)
        v_sb = big.tile([P, NT, HPC, HD + 1], BF16)
        hs_sb = big.tile([P, NT, M], BF16)
        hsT_sb = big.tile([P, 2, NT, P], BF16)
        tri_sb = big.tile([P, P], BF16)
        ident_sb = big.tile([P, P], BF16)
        zz_sb = big.tile([1, 512], BF16)

        nc.vector.memset(v_sb[:, :, :, HD : HD + 1], 1.0)
        nc.vector.memset(zz_sb[:], 0.0)

        # warm-up operand (see `warm` in the pipeline below)
        warm_sb = big.tile([P, 512], BF16)
        nc.vector.memset(warm_sb[:], 0.0)

        # ---- input DMAs ----
        # Two HWDGE rings (sync + scalar sequencers): each dma_start costs
        # ~750ns of serialized descriptor setup on its ring and then runs on
        # its own queue at ~1/(60ns + bytes/30GBps) per 1-partition
        # descriptor.  So: priority order, alternate rings, split the
        # critical pieces by partition halves for latency, keep descriptors
        # >= 2KB.  ScalarE is idle this early, so using its ring is free.
        _rr = [nc.sync, nc.scalar]

        def dma(i, dst, src):
            _rr[i % 2].dma_start(dst, src)

        n = 0
        halves = ((0, 64), (64, 128))

        def wpieces(w_sb, w_d):
            nonlocal n
            for lo, hi in halves:
                for k0 in (0, 4):
                    dma(
                        n,
                        w_sb[lo:hi, k0 : k0 + 4, :],
                        w_d.ap()[lo:hi, M * k0 : M * (k0 + 4)],
                    )
                    n += 1

        # Wave 1: EXACTLY 16 pieces (one per DMA engine; both rings share
        # the 16 engines): wq + wk + xT first-half, everything the q/k
        # projections — and therefore the critical exp chain — need.  All
        # land ~12.5us.  wv rides wave 2: PV work trails the exp chain by
        # up to the e-buffer depth, so late v is harmless.
        wpieces(wq_sb, wq_d)
        wpieces(wk_sb, wk_d)
        for kc in range(KC):
            dma(
                n,
                xT_sb[:, kc, 0:1024],
                xT_d.ap()[P * kc : P * (kc + 1), 0:1024],
            )
            n += 1
        wpieces(wv_sb, wv_d)
        dma(n, bq_sb[:], bq_d.ap().rearrange("(h p) -> p h", p=P)); n += 1
        dma(n, bk_sb[:], bk_d.ap().rearrange("(h p) -> p h", p=P)); n += 1
        dma(n, tri_sb[:], tri_d.ap()); n += 1
        dma(n, ident_sb[:], ident_d.ap()); n += 1
        for kc in range(KC):
            dma(
                n,
                xT_sb[:, kc, 1024:2048],
                xT_d.ap()[P * kc : P * (kc + 1), 1024:2048],
            )
            n += 1
        for lo, hi in halves:
            dma(n, wo_sb[lo:hi, :, :], wo_d.ap()[lo:hi, :]); n += 1

        _QK = {
            "q": (wq_sb, qT_sb, bq_sb),
            "k": (wk_sb, kT_sb, bk_sb),
        }

        def proj_one(proj_ps, which, hp, nq, bufs=1):
            w_sb, t_sb, b_sb = _QK[which]
            ps = proj_ps.tile(
                [P, 512], F32, tag="pj", bufs=bufs, name=f"p{which}{hp}{nq}"
            )
            for kc in range(KC):
                nc.tensor.matmul(
                    ps[:],
                    lhsT=w_sb[:, kc, P * hp : P * (hp + 1)],
                    rhs=xT_sb[:, kc, 512 * nq : 512 * (nq + 1)],
                    start=(kc == 0),
                    stop=(kc == KC - 1),
                )
            nc.vector.tensor_scalar_add(
                t_sb[:, hp, 512 * nq : 512 * (nq + 1)],
                ps[:],
                b_sb[:, hp : hp + 1],
            )

        def proj_qk(proj_ps, hp, nqs, bufs=1):
            for nq in nqs:
                proj_one(proj_ps, "q", hp, nq, bufs=bufs)
                proj_one(proj_ps, "k", hp, nq, bufs=bufs)

        def proj_v(proj_ps, sts, bufs=1):
            for st in sts:
                ps = proj_ps.tile([P, M], F32, tag="pj", bufs=bufs, name=f"pv{st}")
                for kc in range(KC):
                    nc.tensor.matmul(
                        ps[:],
                        lhsT=xT_sb[:, kc, P * st : P * (st + 1)],
                        rhs=wv_sb[:, kc, :],
                        start=(kc == 0),
                        stop=(kc == KC - 1),
                    )
                nc.vector.tensor_copy(
                    v_sb[:, st, :, 0:HD],
                    ps[:].rearrange("p (h d) -> p h d", h=HPC),
                )

        def attn_phase(attn_ps, hp, ph, on_jq_done=None):
            """One (head-pair, q-half): strip-pairs over (kt, 512-chunk).

            With on_jq_done, slot jq is normalized eagerly right after its
            last PV matmul (kt == jq) and the callback is invoked so
            transpose/O-proj work can interleave into this phase.
            """
            qlo, qhi = 1024 * ph, 1024 * (ph + 1)
            # slots: t0 = eta0 jq0-6, t1 = eta1 jq0-6, t2 = [eta0 jq7, eta1 jq7]
            hs_tiles = [
                attn_ps.tile([P, 455], F32, tag="hs", bufs=3, name=f"hs{hp}{ph}{i}")
                for i in range(3)
            ]

            def slot(eta, jql):
                if jql < 7:
                    return hs_tiles[eta], 65 * jql
                return hs_tiles[2], 65 * eta
            # chunk-major: finish each 512-wide q-chunk over all its k-strips
            # before the next chunk, so the phase can start as soon as the
            # first qT/kT projection tile of this q-half exists (instead of
            # all of them) and eager normalizations fire much earlier.
            for c0 in range(qlo, qhi, 512):
                for kt in range((c0 + 512) // P):
                    qs = max(c0, P * kt)
                    w = c0 + 512 - qs
                    s_ps = attn_ps.tile(
                        [P, 1024], F32, tag="sc", bufs=2, name=f"sc{hp}{ph}{kt}{c0}"
                    )
                    for eta in range(2):
                        prow = slice(HD * eta, HD * (eta + 1))
                        nc.tensor.matmul(
                            s_ps[:, 512 * eta : 512 * eta + w],
                            lhsT=kT_sb[prow, hp, P * kt : P * (kt + 1)],
                            rhs=qT_sb[prow, hp, qs : qs + w],
                            start=True,
                            stop=True,
                        )
                    e_sb = exp_pool.tile(
                        [P, 1024], BF16, tag="e", name=f"e{kt}{c0}"
                    )
                    pair = s_ps[:].rearrange("p (g f) -> p g f", g=2)[:, :, 0:w]
                    epair = e_sb[:].rearrange("p (g f) -> p g f", g=2)[:, :, 0:w]
                    nc.scalar.activation(epair, pair, EXP, scale=SCALE)
                    if qs == P * kt:  # strip starts at the diagonal block
                        nc.vector.tensor_tensor(
                            e_sb[:].rearrange("p (g f) -> p g f", g=2)[:, :, 0:P],
                            e_sb[:].rearrange("p (g f) -> p g f", g=2)[:, :, 0:P],
                            tri_sb[:]
                            .rearrange("p (o f) -> p o f", o=1)
                            .broadcast_to([P, 2, P]),
                            op=mybir.AluOpType.mult,
                        )
                    for eta in range(2):
                        h = 2 * hp + eta
                        for jq in range(qs // P, (c0 + 512) // P):
                            jql = jq - 8 * ph
                            t, col = slot(eta, jql)
                            # start=True only on each bank's first-ever
                            # write this phase: it clears has_written for
                            # the WHOLE bank, so every other slot's first
                            # write (start=False, bits clear) overwrites.
                            first = kt == 0 and (
                                jql == 0 or (jql == 7 and eta == 0)
                            )
                            nc.tensor.matmul(
                                t[:, col : col + HD + 1],
                                lhsT=e_sb[
                                    :, 512 * eta + P * jq - qs : 512 * eta + P * jq - qs + P
                                ],
                                rhs=v_sb[:, kt, h, :],
                                start=first,
                                stop=(kt == jq),
                                skip_group_check=True,
                            )
                    if on_jq_done is not None and c0 <= P * kt:
                        jql = kt - 8 * ph
                        recip_t = recip_pool.tile(
                            [P, 2], F32, tag="re", bufs=8, name=f"re{hp}{ph}{kt}"
                        )
                        for eta in range(2):
                            h = 2 * hp + eta
                            t, col = slot(eta, jql)
                            nc.vector.reciprocal(
                                recip_t[:, eta : eta + 1],
                                t[:, col + HD : col + HD + 1],
                            )
                            nc.vector.tensor_scalar_mul(
                                hs_sb[:, kt, HD * h : HD * (h + 1)],
                                t[:, col : col + HD],
                                recip_t[:, eta : eta + 1],
                            )
                        on_jq_done(kt)
            if on_jq_done is not None:
                return
            # normalize: batched reciprocal + broadcast multiplies
            recip_t = recip_pool.tile([P, 16], F32, tag="r", name=f"r{hp}{ph}")
            for eta in range(2):
                nc.vector.reciprocal(
                    recip_t[:, 8 * eta : 8 * eta + 7],
                    hs_tiles[eta][:].rearrange("p (s c) -> p s c", c=65)[:, 0:7, HD],
                )
                nc.vector.reciprocal(
                    recip_t[:, 8 * eta + 7 : 8 * eta + 8],
                    hs_tiles[2][:, 65 * eta + HD : 65 * eta + HD + 1],
                )
            for eta in range(2):
                h = 2 * hp + eta
                nc.vector.tensor_tensor(
                    hs_sb[:, 8 * ph : 8 * ph + 7, HD * h : HD * (h + 1)],
                    hs_tiles[eta][:]
                    .rearrange("p (s c) -> p s c", c=65)[:, 0:7, 0:HD],
                    recip_t[:, 8 * eta : 8 * eta + 7]
                    .rearrange("p (s o) -> p s o", o=1)
                    .broadcast_to([P, 7, HD]),
                    op=mybir.AluOpType.mult,
                )
                nc.vector.tensor_scalar_mul(
                    hs_sb[:, 8 * ph + 7, HD * h : HD * (h + 1)],
                    hs_tiles[2][:, 65 * eta : 65 * eta + HD],
                    recip_t[:, 8 * eta + 7 : 8 * eta + 8],
                )

        def transp(out_ps, hp, ph):
            for jq in range(8 * ph, 8 * ph + 8):
                tp = out_ps.tile([P, P], BF16, tag="io", bufs=1, name=f"tp{hp}{jq}")
                nc.tensor.transpose(
                    tp[:], hs_sb[:, jq, P * hp : P * (hp + 1)], ident_sb[:]
                )
                nc.vector.tensor_copy(hsT_sb[:, hp, jq, :], tp[:])

        def oproj(out_ps, st_range, tag="io", bufs=1, alt=False):
            for st in st_range:
                for dc in range(2):
                    ps = out_ps.tile(
                        [P, 512], F32, tag=tag, bufs=bufs, name=f"o{st}{dc}"
                    )
                    for hp in range(2):
                        nc.tensor.matmul(
                            ps[:],
                            lhsT=hsT_sb[:, hp, st, :],
                            rhs=wo_sb[:, hp, 512 * dc : 512 * (dc + 1)],
                            start=(hp == 0),
                            stop=(hp == 1),
                        )
                    o_sb = outcp.tile([P, 512], BF16, tag="o", name=f"oc{st}{dc}")
                    if alt:
                        eng = nc.vector if dc == 0 else nc.scalar
                        if eng is nc.scalar:
                            nc.scalar.copy(o_sb[:], ps[:])
                        else:
                            nc.vector.tensor_copy(o_sb[:], ps[:])
                    else:
                        nc.any.tensor_copy(o_sb[:], ps[:])
                    if st >= NT - 4:
                        # tail tiles: halve the pieces and use both HWDGE
                        # rings (ScalarE's exp work is over by now) so the
                        # final DMA drains ~4x faster
                        for ring, (lo, hi) in zip(
                            (nc.sync, nc.scalar), ((0, 64), (64, 128))
                        ):
                            ring.dma_start(
                                out_d.ap()[
                                    P * st + lo : P * st + hi,
                                    512 * dc : 512 * (dc + 1),
                                ],
                                o_sb[lo:hi, :],
                            )
                    else:
                        nc.sync.dma_start(
                            out_d.ap()[
                                P * st : P * (st + 1), 512 * dc : 512 * (dc + 1)
                            ],
                            o_sb[:],
                        )

        # ---- pipeline ----
        # Boot phase: PE warm-up matmuls interleaved with the first
        # projections.  The HAM clock gate needs ~3.4us of CONTINUOUS PE
        # activity to release 2.4GHz, and re-throttles after idle windows —
        # but the first ~15us are DMA-paced, so dep-free filler matmuls
        # (lower priority than any ready proj work) keep the PE dense.
        # Must use a FULL 128-row stationary: K=1 warm-ups don't register
        # in the activity monitor (measured).
        with tc.tile_pool(name="boot_ps", bufs=1, space="PSUM") as boot_ps:
            for i in range(32):
                t = boot_ps.tile([P, 512], F32, tag="w", bufs=2, name=f"wm{i}")
                nc.tensor.matmul(
                    t[:],
                    lhsT=warm_sb[:, 0:P],
                    rhs=warm_sb[:],
                    start=True,
                    stop=True,
                    skip_group_check=True,
                )
            # ramp q/k projections run double-buffered here (banks are free
            # before the attention pools open): without this, each 8-MM
            # proj group waits the previous group's PSUM-evacuation read
            # (~0.7us PE bubble per tile) during the DMA-paced ramp.
            # v-proj stays OUT of boot so the boot pool closes early (its
            # chains would otherwise hold the attention banks until ~33us);
            # PV consumers can lag v-proj thanks to the deep e-buffer.
            proj_qk(boot_ps, 0, range(2), bufs=2)
        with tc.tile_pool(name="attn_ps", bufs=1, space="PSUM") as attn_ps:  # 7 banks
            with tc.tile_pool(name="proj_ps", bufs=1, space="PSUM") as proj_ps:  # +1
                proj_v(proj_ps, range(8))
                attn_phase(attn_ps, 0, 0)
                proj_qk(proj_ps, 0, range(2, 4))
                proj_v(proj_ps, range(8, NT))
                attn_phase(attn_ps, 0, 1)
                proj_qk(proj_ps, 1, range(4))  # overlaps attention of hp0
            with tc.tile_pool(name="out_ps", bufs=1, space="PSUM") as out_ps:  # +1
                transp(out_ps, 0, 0)
                transp(out_ps, 0, 1)

                def finish_jq(jq):
                    # transpose hp1's freshly-normalized q-tile (light filler)
                    tp = out_ps.tile([P, P], BF16, tag="io", bufs=1, name=f"tpe{jq}")
                    nc.tensor.transpose(
                        tp[:], hs_sb[:, jq, P : 2 * P], ident_sb[:]
                    )
                    nc.vector.tensor_copy(hsT_sb[:, 1, jq, :], tp[:])

                attn_phase(attn_ps, 1, 0, on_jq_done=finish_jq)
                oproj(out_ps, range(0, 8))  # filler during attn(1,1)
                attn_phase(attn_ps, 1, 1, on_jq_done=finish_jq)
        with tc.tile_pool(name="tail_ps", bufs=1, space="PSUM") as tail_ps:
            oproj(tail_ps, range(8, NT), tag="t", bufs=4, alt=True)

    nc.compile()
    return nc


_NC = None


def _get_nc():
    global _NC
    if _NC is None:
        _NC = build_kernel()
    return _NC


def _tri_upper(n=P):
    m = np.zeros((n, n), np.float32)
    iu = np.triu_indices(n, 0)
    m[iu] = 1.0
    return m.astype(ml_dtypes.bfloat16)


def kernel(x, W_Q, W_K, W_V, W_O, b_Q, b_K, b_V, b_O, _trace=False):
    x = np.asarray(x, np.float32)
    W_Q, W_K = np.asarray(W_Q, np.float32), np.asarray(W_K, np.float32)
    W_V, W_O = np.asarray(W_V, np.float32), np.asarray(W_O, np.float32)
    b_Q, b_K = np.asarray(b_Q, np.float32), np.asarray(b_K, np.float32)
    b_V, b_O = np.asarray(b_V, np.float32), np.asarray(b_O, np.float32)

    nc = _get_nc()
    tri = _tri_upper()
    ident = np.eye(P, dtype=np.float32).astype(ml_dtypes.bfloat16)
    xT_b = [np.ascontiguousarray(x[b].T).astype(ml_dtypes.bfloat16) for b in range(B)]

    def _shuf_in(w):  # [D, M] -> [128, KC*M], partition-contiguous kc-major
        return np.ascontiguousarray(
            w.reshape(KC, P, M).transpose(1, 0, 2).reshape(P, KC * M)
        ).astype(ml_dtypes.bfloat16)

    def _shuf_out(w):  # [M, D] -> [128, 2*D], partition-contiguous hp-major
        return np.ascontiguousarray(
            w.reshape(2, P, D).transpose(1, 0, 2).reshape(P, 2 * D)
        ).astype(ml_dtypes.bfloat16)

    in_maps = []
    for core in range(NCORES):
        b, g = core // GROUPS, core % GROUPS
        cols = slice(M * g, M * (g + 1))
        in_maps.append(
            {
                "xT": xT_b[b],
                "wq": _shuf_in(W_Q[:, cols]),
                "wk": _shuf_in(W_K[:, cols]),
                "wv": _shuf_in(W_V[:, cols]),
                "wo": _shuf_out(W_O[cols, :]),
                "bq": np.ascontiguousarray(b_Q[cols]),
                "bk": np.ascontiguousarray(b_K[cols]),
                "tri": tri,
                "ident": ident,
            }
        )
    res = bass_utils.run_bass_kernel_spmd(
        nc, in_maps, core_ids=list(range(NCORES)), trace=_trace
    )
    const_row = (b_V @ W_O + b_O).astype(np.float32)  # exact: sum(softmax)=1
    out = np.zeros((B, S, D), np.float32)
    for b in range(B):
        acc = res.results[b * GROUPS]["out"].astype(np.float64)
        for g in range(1, GROUPS):
            acc = acc + res.results[b * GROUPS + g]["out"]
        out[b] = (acc + const_row).astype(np.float32)
    if _trace:
        kernel.last_results = res
    return out

